# revision 9
# baseline (speedup 1.0000x reference)
"""CubePad Trainium2 kernel (SBUF composition, manual semaphores).

Input  x: [12, 64, 256, 256] f32  (2 cubes x 6 faces, face order F,R,B,L,T,D)
Output y: [12, 64, 258, 258] f32  (1-px border gathered from neighboring faces)

Sharding: channel-parallel across 8 cores (8 channels each); every core holds
all 12 faces so cross-face border gathers stay local. Pure SPMD.

Strategy: compose each padded face in SBUF and write full 1032B rows. HBM
traffic is the floor (~25.4MB in + 25.6MB out per core; ~380 GB/s combined
when the load+store streams overlap), so the schedule keeps both HWDGE
queues streaming the whole run:
  - 6 asymmetric stages (cube x channel group, widths 2,2,4 / 4,2,2): small
    first stage so stores start early, small last stage so the store-only
    tail is short
  - 3-deep buffer rotation: stage s loads wait only on stage s-3 stores
  - T/D faces load first each stage so the DVE->PE->DVE strip-compose chain
    hides under the f0-3 loads; stores gate on fine-grained sems (f0-3 on
    lateral weaves, f4/5 + col strips on the transpose chain, row strips on
    the Pool strip pass)
  - per-stage-per-group semaphores: every wait covers the complete inc set
    of its semaphore, so DMA completion counting is exact (no cross-stage /
    cross-ring attribution races)
  - straight strips are loaded to SBUF and stored as full 1032B rows
    (replaces half-rate DRAM->DRAM descriptors + 4B corner descriptors)
  - stage-0 loads and late-stage stores are split across both HWDGE rings
    (sync + scalar) so single-direction phases stay descriptor-fed

Raw bass engine programs:
  SP:   loads (stage-0 T/D+strips, stages 1-5 all), stage-4/5 store
        groups B/C
  ACT:  stage-0 f0-3/STR loads, store groups A (all stages), B/C stages 0-3
  Pool: ident (sem_I), RVSo reversal, lateral border weaves (sem_P),
        STRo edge replication (sem_S)
  DVE:  CRS transpose staging (sem_F), PSUM->row/border compose (sem_D)
  PE:   8 transposes per stage (sem_T)
"""

import numpy as np

N_CORES = 8
NF, C_FULL, H, W = 12, 64, 256, 256
C = C_FULL // N_CORES  # 8 channels per core
HP, WP = H + 2, W + 2
NCMAX = 4
PAR = 3  # buffer parities
# stages: (face base, ch lo, ch hi)
STAGE_DEFS = [(0, 0, 2), (0, 2, 4), (0, 4, 8), (6, 0, 4), (6, 4, 6), (6, 6, 8)]
STAGES = len(STAGE_DEFS)
N_TD = 4   # sem_TD incs (x16) per stage: T/D interior loads
N_SP = 8   # sem_SP incs: RCstr/RCs/RVS strip loads
N_FL = 8   # sem_FL incs: f0-3 interior loads
N_SR = 4   # sem_SR incs: STRo straight-strip loads
WRITES = 24  # sem_wr incs (x16) per stage


def _build_bass():
    import concourse.bass as bass
    import concourse.mybir as mybir

    f32 = mybir.dt.float32
    nc = bass.Bass()
    x = nc.dram_tensor("x", [NF, C, H, W], f32, kind="ExternalInput")
    y = nc.dram_tensor("y", [NF, C, HP, WP], f32, kind="ExternalOutput")

    def sb(name, shape):
        return nc.alloc_sbuf_tensor(name, shape, f32)

    ident = sb("ident", [128, 128])
    Y0 = [[sb(f"y0_{p}_{f}", [128, NCMAX, WP]) for f in range(6)]
          for p in range(PAR)]
    Y1 = [[sb(f"y1_{p}_{f}", [128, NCMAX, WP]) for f in range(6)]
          for p in range(PAR)]
    RVS = [sb(f"rvs_{p}", [4 * NCMAX, W]) for p in range(PAR)]
    RVSo = [sb(f"rvso_{p}", [4 * NCMAX, WP]) for p in range(PAR)]
    RCstr = [sb(f"rcstr_{p}", [2 * NCMAX, W]) for p in range(PAR)]
    RCs = [sb(f"rcs_{p}", [2 * NCMAX, W]) for p in range(PAR)]
    RCrev = [sb(f"rcrev_{p}", [2 * NCMAX, W]) for p in range(PAR)]
    STRo = [sb(f"stro_{p}", [4 * NCMAX, WP]) for p in range(PAR)]
    CRSa = [[sb(f"crsa{h}_{p}", [128, 2 * NCMAX]) for h in range(2)]
            for p in range(PAR)]
    CRSb = [[sb(f"crsb{h}_{p}", [128, 2 * NCMAX]) for h in range(2)]
            for p in range(PAR)]
    PSo = [sb(f"pso_{p}", [2 * NCMAX, WP]) for p in range(PAR)]
    PSor = [sb(f"psor_{p}", [2 * NCMAX, WP]) for p in range(PAR)]

    def ps(name, shape):
        return nc.alloc_psum_tensor(name, shape, f32)

    PSa = [ps(f"psa{h}", [2 * NCMAX, 128]) for h in range(2)]
    PSb = [ps(f"psb{h}", [2 * NCMAX, 128]) for h in range(2)]
    PRs = [ps(f"prs{h}", [128, 2 * NCMAX]) for h in range(2)]
    PRr = [ps(f"prr{h}", [128, 2 * NCMAX]) for h in range(2)]

    # per-stage DMA-completion semaphores (exact counting: each wait covers
    # the complete inc set of its sem)
    sem_TD = [nc.alloc_semaphore(f"sem_TD{s}") for s in range(STAGES)]
    sem_SP = [nc.alloc_semaphore(f"sem_SP{s}") for s in range(STAGES)]
    sem_FL = [nc.alloc_semaphore(f"sem_FL{s}") for s in range(STAGES)]
    sem_SR = [nc.alloc_semaphore(f"sem_SR{s}") for s in range(STAGES)]
    sem_wr = [nc.alloc_semaphore(f"sem_wr{s}") for s in range(STAGES)]
    # compute-progress semaphores (single-engine, inc by 1 per stage)
    sem_I = nc.alloc_semaphore("sem_I")
    sem_F = nc.alloc_semaphore("sem_F")
    sem_T = nc.alloc_semaphore("sem_T")
    sem_D = nc.alloc_semaphore("sem_D")
    sem_P = nc.alloc_semaphore("sem_P")
    sem_S = nc.alloc_semaphore("sem_S")

    def sparams(s):
        b, c0, c1 = STAGE_DEFS[s]
        return (s % PAR, b, slice(c0, c1), c1 - c0)

    # ---- load issue helpers (each dma_start incs its group sem by 16) ----
    def loads_td_strips(eng, s):
        p, b, cs, n = sparams(s)
        for f in (4, 5):
            eng.dma_start(
                Y0[p][f][:, 0:n, 1:257],
                x[b + f, cs, 0:128, :].transpose([1, 0, 2]),
            ).then_inc(sem_TD[s], 16)
            eng.dma_start(
                Y1[p][f][:, 0:n, 1:257],
                x[b + f, cs, 128:256, :].transpose([1, 0, 2]),
            ).then_inc(sem_TD[s], 16)
        # row->col sources: straight L r0 / R r255; to-reverse L r255 / R r0
        eng.dma_start(RCstr[p][0:n, :], x[b + 3, cs, 0, :]).then_inc(sem_SP[s], 16)
        eng.dma_start(RCstr[p][4:4 + n, :],
                      x[b + 1, cs, 255, :]).then_inc(sem_SP[s], 16)
        eng.dma_start(RCs[p][0:n, :], x[b + 3, cs, 255, :]).then_inc(sem_SP[s], 16)
        eng.dma_start(RCs[p][4:4 + n, :],
                      x[b + 1, cs, 0, :]).then_inc(sem_SP[s], 16)
        # W-reversed row strips: [0:4]=(B,t)<-T r0, [4:8]=(B,d)<-D r255,
        # [8:12]=(T,t)<-B r0, [12:16]=(D,d)<-B r255
        eng.dma_start(RVS[p][0:n, :], x[b + 4, cs, 0, :]).then_inc(sem_SP[s], 16)
        eng.dma_start(RVS[p][4:4 + n, :],
                      x[b + 5, cs, 255, :]).then_inc(sem_SP[s], 16)
        eng.dma_start(RVS[p][8:8 + n, :],
                      x[b + 2, cs, 0, :]).then_inc(sem_SP[s], 16)
        eng.dma_start(RVS[p][12:12 + n, :],
                      x[b + 2, cs, 255, :]).then_inc(sem_SP[s], 16)

    def loads_ring_str(eng, s):
        p, b, cs, n = sparams(s)
        for f in range(4):
            eng.dma_start(
                Y0[p][f][:, 0:n, 1:257],
                x[b + f, cs, 0:128, :].transpose([1, 0, 2]),
            ).then_inc(sem_FL[s], 16)
            eng.dma_start(
                Y1[p][f][:, 0:n, 1:257],
                x[b + f, cs, 128:256, :].transpose([1, 0, 2]),
            ).then_inc(sem_FL[s], 16)
        # straight strips: [0:4]=(F,t)<-T r255, [4:8]=(D,t)<-F r255,
        # [8:12]=(F,d)<-D r0, [12:16]=(T,d)<-F r0
        eng.dma_start(STRo[p][0:n, 1:257],
                      x[b + 4, cs, 255, :]).then_inc(sem_SR[s], 16)
        eng.dma_start(STRo[p][4:4 + n, 1:257],
                      x[b + 0, cs, 255, :]).then_inc(sem_SR[s], 16)
        eng.dma_start(STRo[p][8:8 + n, 1:257],
                      x[b + 5, cs, 0, :]).then_inc(sem_SR[s], 16)
        eng.dma_start(STRo[p][12:12 + n, 1:257],
                      x[b + 0, cs, 0, :]).then_inc(sem_SR[s], 16)

    # ---- store issue helpers (each dma_start incs sem_wr[s] by 16) ----
    def stores_a(eng, s):  # f0-3 big blocks; needs sem_P >= s+1
        p, b, cs, n = sparams(s)
        for f in range(4):
            eng.dma_start(
                y[b + f, cs, 1:129, :].transpose([1, 0, 2]),
                Y0[p][f][:, 0:n, :]).then_inc(sem_wr[s], 16)
            eng.dma_start(
                y[b + f, cs, 129:257, :].transpose([1, 0, 2]),
                Y1[p][f][:, 0:n, :]).then_inc(sem_wr[s], 16)

    def stores_b(eng, s):  # f4/5 blocks + col-sourced strips; sem_D >= s+1
        p, b, cs, n = sparams(s)
        for f in (4, 5):
            eng.dma_start(
                y[b + f, cs, 1:129, :].transpose([1, 0, 2]),
                Y0[p][f][:, 0:n, :]).then_inc(sem_wr[s], 16)
            eng.dma_start(
                y[b + f, cs, 129:257, :].transpose([1, 0, 2]),
                Y1[p][f][:, 0:n, :]).then_inc(sem_wr[s], 16)
        # col-sourced strips: (L,t), (R,d), (R,t), (L,d)
        eng.dma_start(y[b + 3, cs, 0, :],
                      PSo[p][0:n, :]).then_inc(sem_wr[s], 16)
        eng.dma_start(y[b + 1, cs, 257, :],
                      PSo[p][4:4 + n, :]).then_inc(sem_wr[s], 16)
        eng.dma_start(y[b + 1, cs, 0, :],
                      PSor[p][0:n, :]).then_inc(sem_wr[s], 16)
        eng.dma_start(y[b + 3, cs, 257, :],
                      PSor[p][4:4 + n, :]).then_inc(sem_wr[s], 16)

    def stores_c(eng, s):  # reversed + straight row strips; sem_S >= s+1
        p, b, cs, n = sparams(s)
        eng.dma_start(y[b + 2, cs, 0, :],
                      RVSo[p][0:n, :]).then_inc(sem_wr[s], 16)
        eng.dma_start(y[b + 2, cs, 257, :],
                      RVSo[p][4:4 + n, :]).then_inc(sem_wr[s], 16)
        eng.dma_start(y[b + 4, cs, 0, :],
                      RVSo[p][8:8 + n, :]).then_inc(sem_wr[s], 16)
        eng.dma_start(y[b + 5, cs, 257, :],
                      RVSo[p][12:12 + n, :]).then_inc(sem_wr[s], 16)
        eng.dma_start(y[b + 0, cs, 0, :],
                      STRo[p][0:n, :]).then_inc(sem_wr[s], 16)
        eng.dma_start(y[b + 5, cs, 0, :],
                      STRo[p][4:4 + n, :]).then_inc(sem_wr[s], 16)
        eng.dma_start(y[b + 0, cs, 257, :],
                      STRo[p][8:8 + n, :]).then_inc(sem_wr[s], 16)
        eng.dma_start(y[b + 4, cs, 257, :],
                      STRo[p][12:12 + n, :]).then_inc(sem_wr[s], 16)

    with nc.Block() as block:

        @block.sync
        def _(sp):
            for s in range(STAGES):
                if s >= PAR:
                    sp.wait_ge(sem_wr[s - PAR], WRITES * 16)
                loads_td_strips(sp, s)
                if s > 0:  # stage 0's ring/STR loads issue on scalar
                    loads_ring_str(sp, s)
            # tail: stage-4/5 store groups B and C on the sync ring
            for s in (STAGES - 2, STAGES - 1):
                sp.wait_ge(sem_D, s + 1)
                stores_b(sp, s)
                sp.wait_ge(sem_S, s + 1)
                stores_c(sp, s)

        @block.scalar
        def _(ac):
            loads_ring_str(ac, 0)
            for s in range(STAGES):
                ac.wait_ge(sem_P, s + 1)
                stores_a(ac, s)
                if s < STAGES - 2:
                    ac.wait_ge(sem_D, s + 1)
                    stores_b(ac, s)
                    ac.wait_ge(sem_S, s + 1)
                    stores_c(ac, s)

        @block.gpsimd
        def _(gp):
            gp.memset(ident[:, :], 0.0)
            gp.drain()
            gp.affine_select(
                out=ident[:, :], in_=ident[:, :],
                compare_op=mybir.AluOpType.not_equal, fill=1.0, base=0,
                pattern=[[-1, 128]], channel_multiplier=1,
            ).then_inc(sem_I, 1)
            for s in range(STAGES):
                p, b, cs, n = sparams(s)
                gp.wait_ge(sem_SP[s], N_SP * 16)
                gp.tensor_copy(RVSo[p][:, 1:257], RVS[p][:, ::-1])
                gp.drain()
                gp.tensor_copy(RVSo[p][:, 0:1], RVSo[p][:, 1:2])
                gp.tensor_copy(RVSo[p][:, 257:258], RVSo[p][:, 256:257])
                # lateral borders: l: F<-L,R<-F,B<-R,L<-B (x col 255 = col
                # 256); r: F<-R,R<-B,B<-L,L<-F (x col 0 = col 1)
                gp.wait_ge(sem_TD[s], N_TD * 16)
                gp.wait_ge(sem_FL[s], N_FL * 16)
                for Yh in (Y0[p], Y1[p]):
                    for i in range(4):
                        gp.tensor_copy(Yh[i][:, :, 0],
                                       Yh[(i + 3) % 4][:, :, 256])
                        gp.tensor_copy(Yh[i][:, :, 257],
                                       Yh[(i + 1) % 4][:, :, 1])
                gp.drain().then_inc(sem_P, 1)
                gp.wait_ge(sem_SR[s], N_SR * 16)
                gp.tensor_copy(STRo[p][:, 0:1], STRo[p][:, 1:2])
                gp.tensor_copy(STRo[p][:, 257:258], STRo[p][:, 256:257])
                gp.drain().then_inc(sem_S, 1)

        @block.vector
        def _(ve):
            for s in range(STAGES):
                p, b, cs, n = sparams(s)
                ve.wait_ge(sem_TD[s], N_TD * 16)
                # CRS staging: CRSa [0:4]=T c0 (->L,t), [4:8]=D c255 (->R,d)
                #              CRSb [0:4]=T c255 (->R,t), [4:8]=D c0 (->L,d)
                for h, (Yh, CRSah, CRSbh) in enumerate(
                        ((Y0[p], CRSa[p][0], CRSb[p][0]),
                         (Y1[p], CRSa[p][1], CRSb[p][1]))):
                    ve.tensor_copy(CRSah[:, 0:4], Yh[4][:, :, 1])
                    ve.tensor_copy(CRSah[:, 4:8], Yh[5][:, :, 256])
                    ve.tensor_copy(CRSbh[:, 0:4], Yh[4][:, :, 256])
                    ve.tensor_copy(CRSbh[:, 4:8], Yh[5][:, :, 1])
                ve.wait_ge(sem_SP[s], N_SP * 16)
                ve.tensor_copy(RCrev[p][:, :], RCs[p][:, ::-1])
                ve.drain().then_inc(sem_F, 1)
                ve.wait_ge(sem_T, s + 1)
                # composed t/d rows: PSo [0:4]=(L,t), [4:8]=(R,d) straight;
                # PSor [0:4]=(R,t), [4:8]=(L,d) reversed (half-swapped)
                ve.tensor_copy(PSo[p][:, 1:129], PSa[0][:, :])
                ve.tensor_copy(PSo[p][:, 129:257], PSa[1][:, :])
                ve.tensor_copy(PSor[p][:, 1:129], PSb[1][:, ::-1])
                ve.tensor_copy(PSor[p][:, 129:257], PSb[0][:, ::-1])
                ve.drain()
                for t in (PSo[p], PSor[p]):
                    ve.tensor_copy(t[:, 0:1], t[:, 1:2])
                    ve.tensor_copy(t[:, 257:258], t[:, 256:257])
                # T/D left/right borders from PR transposes
                for h, Yh in enumerate((Y0[p], Y1[p])):
                    ve.tensor_copy(Yh[4][:, :, 0], PRs[h][:, 0:4])
                    ve.tensor_copy(Yh[5][:, :, 257], PRs[h][:, 4:8])
                    ve.tensor_copy(Yh[5][:, :, 0], PRr[h][:, 0:4])
                    ve.tensor_copy(Yh[4][:, :, 257], PRr[h][:, 4:8])
                ve.drain().then_inc(sem_D, 1)

        @block.tensor
        def _(te):
            te.wait_ge(sem_I, 1)
            for s in range(STAGES):
                p, b, cs, n = sparams(s)
                te.wait_ge(sem_F, s + 1)
                id8 = ident[0:8, 0:8]
                te.transpose(PSa[0][:, :], CRSa[p][0][:, :], ident[:, :])
                te.transpose(PSa[1][:, :], CRSa[p][1][:, :], ident[:, :])
                te.transpose(PSb[0][:, :], CRSb[p][0][:, :], ident[:, :])
                te.transpose(PSb[1][:, :], CRSb[p][1][:, :], ident[:, :])
                te.transpose(PRs[0][:, :], RCstr[p][:, 0:128], id8)
                te.transpose(PRs[1][:, :], RCstr[p][:, 128:256], id8)
                te.transpose(PRr[0][:, :], RCrev[p][:, 0:128], id8)
                te.transpose(PRr[1][:, :], RCrev[p][:, 128:256],
                             id8).then_inc(sem_T, 1)

    with nc.Block() as block2:

        @block2.sync
        def _(sp):
            for s in range(STAGES):
                sp.wait_ge(sem_wr[s], WRITES * 16)

    nc.finalize()
    return nc


_NC_CACHE = None
_TRACE = False  # set by test.py to collect an NTFF profile
_LAST_EXEC_NS = None


def kernel(x: np.ndarray) -> np.ndarray:
    global _NC_CACHE, _LAST_EXEC_NS
    from concourse.bass_utils import run_bass_kernel_spmd

    assert x.shape == (NF, C_FULL, H, W) and x.dtype == np.float32
    if _NC_CACHE is None:
        _NC_CACHE = _build_bass()
    nc = _NC_CACHE

    in_maps = [
        {"x": np.ascontiguousarray(x[:, i * C:(i + 1) * C])} for i in range(N_CORES)
    ]
    res = run_bass_kernel_spmd(
        nc, in_maps, core_ids=list(range(N_CORES)), trace=_TRACE
    )
    _LAST_EXEC_NS = res.exec_time_ns
    out = np.empty((NF, C_FULL, HP, WP), dtype=np.float32)
    for i in range(N_CORES):
        out[:, i * C:(i + 1) * C] = res.results[i]["y"]
    return out
